# revision 18
# baseline (speedup 1.0000x reference)
"""Trainium2 Bass kernel for nn_ArgExtractorLayer_35527969472569.

ProbSparse (Informer) cross-attention + FFN layer, distributed over 8
NeuronCores: attention is sharded by (batch, head-quad); a pairwise
AllToAll swaps token-halves so the FFN runs token-parallel.

Host-side work is limited to layout (slicing / transposing / dtype
casts) and input-independent constants (the fixed ProbSparse sampling
pattern of jax.random.key(42)); all input-dependent math runs on
device.
"""

import math
import numpy as np
import ml_dtypes

import concourse.bass as bass
import concourse.bacc as bacc
import concourse.tile as tile
from concourse import mybir
from concourse.bass_utils import run_bass_kernel_spmd

L, B, D = 1024, 4, 768
H, DH, DFF = 8, 96, 2048
U = min(int(5 * math.ceil(math.log(L))), L)  # 35
NC = 8
LN_EPS = 1e-5
SCALE = 1.0 / math.sqrt(DH)
QC = L // 128            # 8 query chunks per (b,h)
KC = L // 128            # 8 key chunks
DC = D // 128            # 6 d_model chunks
FC = DFF // 128          # 16 d_ff chunks
TC = 4                   # 4 token chunks of 128 per core (512 tokens)

f32 = mybir.dt.float32
f32r = mybir.dt.float32r
bf16 = mybir.dt.bfloat16
A = mybir.AluOpType
AF = mybir.ActivationFunctionType
X = mybir.AxisListType.X

_bf = ml_dtypes.bfloat16


def _sample_idx():
    """The constant ProbSparse sampling pattern (input-independent)."""
    import jax
    with jax.default_device(jax.devices("cpu")[0]):
        return np.asarray(jax.random.randint(jax.random.key(42), (L, U), 0, L))


def _constants():
    idx = _sample_idx()
    cnt = np.zeros((L, L), np.float32)
    for l in range(L):
        np.add.at(cnt[l], idx[l], 1.0)
    mask = np.where(cnt > 0, 0.0, -1e30).astype(np.float32)
    # mask_h[p, qc, j] = mask[qc*128+p, j]
    mask_h = np.ascontiguousarray(
        mask.reshape(QC, 128, L).transpose(1, 0, 2)).astype(_bf)
    # cntt_h[kc, lc, p, m] = cnt[lc*128+m, kc*128+p]  (CNT^T chunks)
    # cntt_h[p, kc, lc, m] = cnt[lc*128+m, kc*128+p]
    cntt_h = np.ascontiguousarray(
        cnt.T.reshape(KC, 128, QC, 128).transpose(1, 0, 2, 3)).astype(_bf)
    return mask_h, cntt_h


def build(ln1_identity: bool, ln2_identity: bool):
    nc = bacc.Bacc(None, target_bir_lowering=False, num_devices=NC)

    def inp(name, shape, dt):
        return nc.dram_tensor(name, shape, dt, kind="ExternalInput")

    qt_d = inp("qt", [4, DH, L], bf16)
    kt_d = inp("kt", [4, DH, L], bf16)
    qrow_d = inp("qrow", [128, 4, KC, DH], bf16)
    krr_d = inp("krr", [128, 4, KC, DH], bf16)
    krb_d = inp("krb", [128, 4, KC, DH], bf16)
    mask_d = inp("mask", [128, QC, L], bf16)
    cntt_d = inp("cntt", [128, KC, QC, 128], bf16)
    tgt_d = inp("tgt", [128, TC, D], f32)
    w1t_d = inp("w1t", [128, DC, DFF], bf16)
    b1c_d = inp("b1c", [128, FC], f32)
    w2t_d = inp("w2t", [128, FC, D], bf16)
    b2r_d = inp("b2r", [1, D], bf16)
    identb_d = inp("identb", [128, 128], bf16)
    onesr_d = inp("onesr", [1, 128], f32)       # row of ones (f32, exact bcast)
    onescol_d = inp("onescol", [128, 1], bf16)  # column of ones (colsum lhsT)
    onesbf_d = inp("onesbf", [1, 128], bf16)
    identf_d = inp("identf", [128, 128], f32)
    zlo_d = inp("zlo", [128, 1], f32)
    zhi_d = inp("zhi", [128, 1], f32)
    if not ln1_identity:
        g1b_d = inp("g1b", [1, D], f32)
        b1b_d = inp("b1b", [1, D], f32)
    if not ln2_identity:
        g2b_d = inp("g2b", [1, D], f32)
        b2b_d = inp("b2b", [1, D], f32)
    out_d = nc.dram_tensor("out", [128, TC, D], f32, kind="ExternalOutput")

    import contextlib
    with tile.TileContext(nc) as tc, contextlib.ExitStack() as ctx:
        singles = ctx.enter_context(tc.tile_pool(name="singles", bufs=1))
        dram = ctx.enter_context(tc.tile_pool(name="dram", bufs=1, space="DRAM"))

        identb = singles.tile([128, 128], bf16)
        nc.sync.dma_start(out=identb, in_=identb_d[:, :])
        identf = singles.tile([128, 128], f32)
        nc.sync.dma_start(out=identf, in_=identf_d[:, :])
        onesr = singles.tile([1, 128], f32)
        nc.sync.dma_start(out=onesr, in_=onesr_d[:, :])
        onescol = singles.tile([128, 1], bf16)
        nc.sync.dma_start(out=onescol, in_=onescol_d[:, :])
        onesbf = singles.tile([1, 128], bf16)
        nc.sync.dma_start(out=onesbf, in_=onesbf_d[:, :])
        b2r = singles.tile([1, D], bf16)
        nc.sync.dma_start(out=b2r, in_=b2r_d[:, :])
        b1c = singles.tile([128, FC], f32)
        nc.sync.dma_start(out=b1c, in_=b1c_d[:, :])
        w1t = singles.tile([128, DC, DFF], bf16)
        w2t = singles.tile([128, FC, D], bf16)
        krb = singles.tile([128, 4, KC, DH], bf16)
        nc.gpsimd.dma_start(out=krb, in_=krb_d[:, :, :, :])
        mask_t = singles.tile([128, QC, L], bf16)
        nc.gpsimd.dma_start(out=mask_t, in_=mask_d[:, :, :])
        cntt_sb = singles.tile([128, KC, QC, 128], bf16)
        nc.gpsimd.dma_start(out=cntt_sb, in_=cntt_d[:, :, :, :])
        zlo = singles.tile([128, 1], f32)
        nc.sync.dma_start(out=zlo, in_=zlo_d[:, :])
        zhi = singles.tile([128, 1], f32)
        nc.sync.dma_start(out=zhi, in_=zhi_d[:, :])
        epsb = singles.tile([128, 1], f32)
        nc.vector.memset(epsb, LN_EPS)

        ksum = singles.tile([128, QC, 4, DH], f32)   # Ksum = CNT @ K, all pairs
        meanvb = singles.tile([128, 4, DH], f32)     # mean of V rows, bcast

        # RS input shards: [token_half, head_slot 8, l_local, dh]
        ctxbuf = dram.tile([2, 8, 512, DH], bf16)
        rsout = dram.tile([8, 512, DH], bf16)
        m_scr = dram.tile([4, L], f32)

        # ---- Stage 0: Ksum (CNT @ K) and meanV, batched over the 4 pairs
        with tc.tile_pool(name="st0ps", bufs=2, space="PSUM") as st0ps, \
             tc.tile_pool(name="st0ps1", bufs=1, space="PSUM") as st0ps1, \
             tc.tile_pool(name="krrp", bufs=1) as krrp:
            krr = krrp.tile([128, 4, KC, DH], bf16)
            nc.gpsimd.dma_start(out=krr, in_=krr_d[:, :, :, :])

            mv_ps = st0ps1.tile([1, 4, DH], f32)
            for kc in range(KC):
                nc.tensor.matmul(mv_ps, onescol, krr[:, :, kc, :],
                                 start=(kc == 0), stop=(kc == KC - 1))
            mv_sb = singles.tile([1, 4, DH], f32)
            nc.scalar.activation(out=mv_sb, in_=mv_ps, func=AF.Copy,
                                 bias=0.0, scale=1.0 / L)
            mvb_ps = st0ps1.tile([128, 4, DH], f32)
            nc.tensor.matmul(mvb_ps, onesr, mv_sb, start=True, stop=True)
            nc.scalar.copy(meanvb, mvb_ps)

            for lc in range(QC):
                ks_ps = st0ps.tile([128, 4, DH], f32)
                for kc in range(KC):
                    nc.tensor.matmul(ks_ps, cntt_sb[:, kc, lc, :],
                                     krr[:, :, kc, :],
                                     start=(kc == 0), stop=(kc == KC - 1))
                nc.scalar.copy(ksum[:, lc, :, :], ks_ps)

        # ---- Stage 1: per-(b,h) attention
        with tc.tile_pool(name="pqk", bufs=2) as pqk, \
             tc.tile_pool(name="petr", bufs=2) as petr, \
             tc.tile_pool(name="pscr", bufs=2) as pscr, \
             tc.tile_pool(name="psml", bufs=2) as psml, \
             tc.tile_pool(name="pctx", bufs=2) as pctx, \
             tc.tile_pool(name="psA", bufs=2, space="PSUM") as psA, \
             tc.tile_pool(name="psU", bufs=2, space="PSUM") as psU:
            for p in range(4):
                qt_t = pqk.tile([DH, L], bf16, tag="qt")
                nc.scalar.dma_start(out=qt_t, in_=qt_d[p, :, :])
                kt_t = pqk.tile([DH, L], bf16, tag="kt")
                nc.scalar.dma_start(out=kt_t, in_=kt_d[p, :, :])
                qrow_t = pqk.tile([128, KC, DH], bf16, tag="qrow")
                nc.sync.dma_start(out=qrow_t, in_=qrow_d[:, p, :, :])

                etr = petr.tile([128, KC, L], bf16, tag="etr")   # E^T chunks
                mmax_t = psml.tile([128, QC], f32, tag="mmax")
                msum_t = psml.tile([128, QC], f32, tag="msum")
                esum_t = psml.tile([128, QC], f32, tag="esum")

                # S chunks: M-measure stats + softmax row sums
                for qc in range(QC):
                    s_ps = psA.tile([128, L], f32, tag="S")
                    for h in range(2):
                        nc.tensor.matmul(
                            s_ps[:, h * 512:(h + 1) * 512],
                            qt_t[:, qc * 128:(qc + 1) * 128],
                            kt_t[:, h * 512:(h + 1) * 512],
                            start=True, stop=True)
                    scr = pscr.tile([128, L], bf16, tag="scr")
                    nc.vector.tensor_tensor(scr, s_ps, mask_t[:, qc, :], A.add)
                    nc.vector.reduce_max(mmax_t[:, qc:qc + 1], scr, axis=X)
                    escr = pscr.tile([128, L], bf16, tag="escr")
                    nc.scalar.activation(out=escr, in_=s_ps,
                                         func=AF.Exp, bias=0.0, scale=SCALE,
                                         accum_out=esum_t[:, qc:qc + 1])
                    scr96 = pscr.tile([128, DH], f32, tag="scr96")
                    nc.vector.scalar_tensor_tensor(
                        out=scr96, in0=qrow_t[:, qc, :], scalar=1.0,
                        in1=ksum[:, qc, p, :], op0=A.bypass, op1=A.mult,
                        accum_out=msum_t[:, qc:qc + 1])

                # S^T chunks -> E^T directly (no PE transposes of E)
                for kc in range(KC):
                    st_ps = psA.tile([128, L], f32, tag="S")
                    for h in range(2):
                        nc.tensor.matmul(
                            st_ps[:, h * 512:(h + 1) * 512],
                            kt_t[:, kc * 128:(kc + 1) * 128],
                            qt_t[:, h * 512:(h + 1) * 512],
                            start=True, stop=True)
                    nc.scalar.activation(out=etr[:, kc, :], in_=st_ps,
                                         func=AF.Exp, bias=0.0, scale=SCALE)

                # M = mmax - msum/U ; exact f32 broadcast ; rank count
                m_t = psml.tile([128, QC], f32, tag="m")
                nc.vector.scalar_tensor_tensor(
                    out=m_t, in0=msum_t, scalar=-1.0 / U, in1=mmax_t,
                    op0=A.mult, op1=A.add)
                mdst = bass.AP(tensor=m_scr.tensor,
                               offset=m_scr.offset + p * L,
                               ap=[[1, 128], [128, QC]])
                nc.sync.dma_start(out=mdst, in_=m_t)
                mrow = psml.tile([1, L], f32, tag="mrow")
                nc.sync.dma_start(out=mrow, in_=m_scr[p, None, :])
                mb_ps = psA.tile([128, L], f32, tag="S")
                for h in range(2):
                    nc.tensor.matmul(mb_ps[:, h * 512:(h + 1) * 512], onesr,
                                     mrow[:, h * 512:(h + 1) * 512],
                                     start=True, stop=True)
                mb_sb = pscr.tile([128, L], f32, tag="mb")
                nc.scalar.copy(mb_sb, mb_ps)
                cnt_t = psml.tile([128, QC], f32, tag="cnt")
                for qc in range(QC):
                    scr2 = pscr.tile([128, L], bf16, tag="rcscr")
                    nc.vector.tensor_scalar(
                        out=scr2, in0=mb_sb, scalar1=m_t[:, qc:qc + 1],
                        scalar2=None, op0=A.is_gt, op1=A.add,
                        accum_out=cnt_t[:, qc:qc + 1])
                sel_t = psml.tile([128, QC], f32, tag="sel")
                nc.vector.tensor_scalar(out=sel_t, in0=cnt_t, scalar1=float(U),
                                        scalar2=None, op0=A.is_lt)
                nsel_t = psml.tile([128, QC], f32, tag="nsel")
                nc.vector.tensor_scalar(out=nsel_t, in0=sel_t, scalar1=-1.0,
                                        scalar2=-1.0, op0=A.mult, op1=A.subtract)
                recip_t = psml.tile([128, QC], f32, tag="recip")
                nc.vector.reciprocal(recip_t, esum_t)
                a_t = psml.tile([128, QC], f32, tag="a")
                nc.vector.tensor_tensor(a_t, sel_t, recip_t, A.mult)

                # PV in transposed orientation: updT[dh, l] accumulated over kc
                updt_full = psA.tile([128, L], f32, tag="S")
                updt_ps = updt_full[0:DH, :]
                for kc in range(KC):
                    for h in range(2):
                        nc.tensor.matmul(
                            updt_ps[:, h * 512:(h + 1) * 512],
                            krb[:, p, kc, :],
                            etr[:, kc, h * 512:(h + 1) * 512],
                            start=(kc == 0), stop=(kc == KC - 1))
                updt_sb = pscr.tile([DH, L], bf16, tag="updt")
                nc.scalar.copy(updt_sb, updt_ps)

                ctx_sb = pctx.tile([128, QC, DH], bf16, tag="ctx")
                for qc in range(QC):
                    u_ps = psU.tile([128, DH], bf16, tag="u")
                    nc.tensor.transpose(u_ps,
                                        updt_sb[:, qc * 128:(qc + 1) * 128],
                                        identb[0:DH, 0:DH])
                    u1 = pscr.tile([128, DH], f32, tag="u1")
                    nc.scalar.activation(out=u1, in_=u_ps, func=AF.Identity,
                                         bias=0.0, scale=a_t[:, qc:qc + 1])
                    nc.vector.scalar_tensor_tensor(
                        out=ctx_sb[:, qc, :], in0=meanvb[:, p, :],
                        scalar=nsel_t[:, qc:qc + 1], in1=u1,
                        op0=A.mult, op1=A.add)

                # masked copies: head slot p (low quad) and 4+p (high quad)
                ctx_lo = pctx.tile([128, QC, DH], bf16, tag="ctxlo")
                nc.vector.tensor_scalar(out=ctx_lo, in0=ctx_sb,
                                        scalar1=zlo[:, 0:1], scalar2=None,
                                        op0=A.mult)
                ctx_hi = pctx.tile([128, QC, DH], bf16, tag="ctxhi")
                nc.vector.tensor_scalar(out=ctx_hi, in0=ctx_sb,
                                        scalar1=zhi[:, 0:1], scalar2=None,
                                        op0=A.mult)
                for tl, slot in ((ctx_lo, p), (ctx_hi, 4 + p)):
                    for m in range(2):
                        cdst = bass.AP(
                            tensor=ctxbuf.tensor,
                            offset=(ctxbuf.offset + m * 8 * 512 * DH
                                    + slot * 512 * DH),
                            ap=[[DH, 128], [128 * DH, 4], [1, DH]])
                        nc.sync.dma_start(out=cdst,
                                          in_=tl[:, m * 4:(m + 1) * 4, :])

        # ---- Stage 2: pairwise ReduceScatter (heads are feature-disjoint,
        # so the add concatenates this core's quad with its partner's).
        nc.gpsimd.collective_compute(
            "ReduceScatter", A.add,
            replica_groups=[[0, 1], [2, 3], [4, 5], [6, 7]],
            ins=[ctxbuf.opt()], outs=[rsout.opt()])
        # rsout[h, l_local, dh]: all 8 heads for MY 512 tokens.

        # ---- Stage 3: token-parallel LN1 -> FFN -> LN2
        with tc.tile_pool(name="ffn", bufs=2) as ffn, \
             tc.tile_pool(name="ffn1", bufs=1) as ffn1, \
             tc.tile_pool(name="ptiny", bufs=4) as ptiny, \
             tc.tile_pool(name="psN", bufs=1, space="PSUM") as psN, \
             tc.tile_pool(name="psH", bufs=2, space="PSUM") as psH, \
             tc.tile_pool(name="psP", bufs=2, space="PSUM") as psP:
            nc.scalar.dma_start(out=w1t, in_=w1t_d[:, :, :])
            nc.scalar.dma_start(out=w2t, in_=w2t_d[:, :, :])
            n1_all = ffn1.tile([128, TC, D], f32)
            n1t_sb = ffn1.tile([128, DC, TC, 128], bf16)
            h1t_sb = ffn1.tile([128, FC, 512], bf16)
            tgt_t = ffn1.tile([128, TC, D], f32)
            nc.gpsimd.dma_start(out=tgt_t, in_=tgt_d[:, :, :])
            gb1 = bb1 = gb2 = bb2 = None
            if not ln1_identity:
                gb1 = ffn1.tile([128, D], f32)
                nc.sync.dma_start(out=gb1, in_=g1b_d[:, :].broadcast_to([128, D]))
                bb1 = ffn1.tile([128, D], f32)
                nc.sync.dma_start(out=bb1, in_=b1b_d[:, :].broadcast_to([128, D]))
            if not ln2_identity:
                gb2 = ffn1.tile([128, D], f32)
                nc.sync.dma_start(out=gb2, in_=g2b_d[:, :].broadcast_to([128, D]))
                bb2 = ffn1.tile([128, D], f32)
                nc.sync.dma_start(out=bb2, in_=b2b_d[:, :].broadcast_to([128, D]))

            def layer_norm(x_t, out_ap, gb, bb):
                """LN over the free dim (D) with per-partition stats."""
                sum_t = ptiny.tile([128, 1], f32, tag="t1")
                nc.vector.reduce_sum(sum_t, x_t, axis=X)
                sq_scr = ffn.tile([128, D], f32, tag="sqscr")
                ssq_t = ptiny.tile([128, 1], f32, tag="t2")
                nc.scalar.activation(out=sq_scr, in_=x_t, func=AF.Square,
                                     bias=0.0, scale=1.0, accum_out=ssq_t)
                mu_t = ptiny.tile([128, 1], f32, tag="t3")
                nc.vector.tensor_scalar(out=mu_t, in0=sum_t, scalar1=1.0 / D,
                                        scalar2=None, op0=A.mult)
                musq_t = ptiny.tile([128, 1], f32, tag="t4")
                nc.vector.tensor_tensor(musq_t, mu_t, mu_t, A.mult)
                var_t = ptiny.tile([128, 1], f32, tag="t5")
                nc.vector.scalar_tensor_tensor(
                    out=var_t, in0=ssq_t, scalar=1.0 / D, in1=musq_t,
                    op0=A.mult, op1=A.subtract)
                sd_t = ptiny.tile([128, 1], f32, tag="t6")
                nc.scalar.activation(out=sd_t, in_=var_t, func=AF.Sqrt,
                                     bias=epsb[:, 0:1], scale=1.0)
                rstd_t = ptiny.tile([128, 1], f32, tag="t7")
                nc.vector.reciprocal(rstd_t, sd_t)
                nb_t = ptiny.tile([128, 1], f32, tag="t8")
                nc.vector.scalar_tensor_tensor(
                    out=nb_t, in0=mu_t, scalar=-1.0, in1=rstd_t,
                    op0=A.mult, op1=A.mult)
                if gb is None:
                    nc.scalar.activation(out=out_ap, in_=x_t, func=AF.Identity,
                                         bias=nb_t, scale=rstd_t)
                else:
                    xh = ffn.tile([128, D], f32, tag="xhat")
                    nc.scalar.activation(out=xh, in_=x_t, func=AF.Identity,
                                         bias=nb_t, scale=rstd_t)
                    xg = ffn.tile([128, D], f32, tag="xg")
                    nc.vector.tensor_tensor(xg, xh, gb, A.mult)
                    nc.vector.tensor_tensor(out_ap, xg, bb, A.add)

            for tc4 in range(TC):
                att_t = ffn.tile([128, D], bf16, tag="att")
                asrc = bass.AP(tensor=rsout.tensor,
                               offset=rsout.offset + tc4 * 128 * DH,
                               ap=[[DH, 128], [512 * DH, 8], [1, DH]])
                nc.sync.dma_start(out=att_t, in_=asrc)
                x_t = ffn.tile([128, D], f32, tag="x")
                nc.vector.tensor_tensor(x_t, tgt_t[:, tc4, :], att_t, A.add)
                layer_norm(x_t, n1_all[:, tc4, :], gb1, bb1)
                n1t_ps = psN.tile([128, DC, 128], f32, tag="n1t")
                for dc in range(DC):
                    nc.tensor.transpose(n1t_ps[:, dc, :],
                                        n1_all[:, tc4, dc * 128:(dc + 1) * 128],
                                        identf)
                nc.scalar.copy(n1t_sb[:, :, tc4, :], n1t_ps)

            for fc in range(FC):
                h_ps = psH.tile([128, 512], f32, tag="h")
                for dc in range(DC):
                    nc.tensor.matmul(h_ps, w1t[:, dc, fc * 128:(fc + 1) * 128],
                                     n1t_sb[:, dc, :, :],
                                     start=(dc == 0), stop=(dc == DC - 1))
                nc.scalar.activation(out=h1t_sb[:, fc, :], in_=h_ps, func=AF.Relu,
                                     bias=b1c[:, fc:fc + 1], scale=1.0)

            for tc4 in range(TC):
                pr_ps = psP.tile([128, D], f32, tag="pr")
                for lo, hi in ((0, 512), (512, D)):
                    for fc in range(FC):
                        nc.tensor.matmul(pr_ps[:, lo:hi],
                                         h1t_sb[:, fc, tc4 * 128:(tc4 + 1) * 128],
                                         w2t[:, fc, lo:hi],
                                         start=(fc == 0), stop=False)
                    nc.tensor.matmul(pr_ps[:, lo:hi], onesbf, b2r[:, lo:hi],
                                     start=False, stop=True)
                x2_t = ffn.tile([128, D], f32, tag="x2")
                nc.vector.tensor_tensor(x2_t, n1_all[:, tc4, :], pr_ps, A.add)
                out_sb = ffn.tile([128, D], f32, tag="osb")
                layer_norm(x2_t, out_sb, gb2, bb2)
                nc.sync.dma_start(out=out_d[:, tc4, :], in_=out_sb)

    nc.finalize()
    return nc


# ---------------------------------------------------------------------------
# Host-side sharding + execution

_cache = {}


def _get_built(ln1_identity, ln2_identity):
    key = (ln1_identity, ln2_identity)
    if key not in _cache:
        _cache[key] = build(ln1_identity, ln2_identity)
    return _cache[key]


def _shard_inputs(target, source, W1, b1, W2, b2, ln1_g, ln1_b, ln2_g, ln2_b):
    mask_h, cntt_h = _constants()
    identb = np.eye(128, dtype=np.float32).astype(_bf)
    identf = np.eye(128, dtype=np.float32)
    onesr = np.ones((1, 128), np.float32)
    onescol = np.ones((128, 1), np.float32).astype(_bf)
    onesbf = np.ones((1, 128), np.float32).astype(_bf)
    w1t = np.ascontiguousarray(W1.T.reshape(DC, 128, DFF).transpose(1, 0, 2)).astype(_bf)
    b1c = np.ascontiguousarray(b1.reshape(FC, 128).T).astype(np.float32)
    w2t = np.ascontiguousarray(W2.T.reshape(FC, 128, D).transpose(1, 0, 2)).astype(_bf)
    b2r = b2.reshape(1, D).astype(_bf)

    # target/source as [B, H, L, Dh] views
    tr = target.reshape(L, B, H, DH)
    sr = source.reshape(L, B, H, DH)

    ln1_identity = bool(np.all(ln1_g == 1.0) and np.all(ln1_b == 0.0))
    ln2_identity = bool(np.all(ln2_g == 1.0) and np.all(ln2_b == 0.0))

    in_maps = []
    for c in range(NC):
        b = c // 2
        hq = c % 2
        lh = c % 2
        hs = slice(4 * hq, 4 * hq + 4)
        q = np.ascontiguousarray(tr[:, b, hs, :])      # [L, 4, DH]
        k = np.ascontiguousarray(sr[:, b, hs, :])      # [L, 4, DH]
        qt = np.ascontiguousarray(q.transpose(1, 2, 0)).astype(_bf)   # [4, DH, L]
        kt = np.ascontiguousarray(k.transpose(1, 2, 0)).astype(_bf)   # [4, DH, L]
        # [128, 4, KC, DH]: [pp, pair, kc, dh] with l = kc*128+pp
        qrow = np.ascontiguousarray(
            q.reshape(KC, 128, 4, DH).transpose(1, 2, 0, 3))
        krow = np.ascontiguousarray(
            k.reshape(KC, 128, 4, DH).transpose(1, 2, 0, 3))
        tgt = np.ascontiguousarray(
            target[lh * 512:(lh + 1) * 512, b, :]
            .reshape(TC, 128, D).transpose(1, 0, 2))
        zlo = np.full((128, 1), 1.0 if hq == 0 else 0.0, np.float32)
        zhi = np.full((128, 1), 0.0 if hq == 0 else 1.0, np.float32)
        m = dict(qt=qt, kt=kt, qrow=qrow.astype(_bf), krr=krow.astype(_bf),
                 krb=krow.astype(_bf),
                 mask=mask_h, cntt=cntt_h, tgt=tgt, w1t=w1t, b1c=b1c,
                 w2t=w2t, b2r=b2r, identb=identb, identf=identf,
                 onesr=onesr, onescol=onescol, onesbf=onesbf,
                 zlo=zlo, zhi=zhi)
        if not ln1_identity:
            m["g1b"] = ln1_g.reshape(1, D).astype(np.float32)
            m["b1b"] = ln1_b.reshape(1, D).astype(np.float32)
        if not ln2_identity:
            m["g2b"] = ln2_g.reshape(1, D).astype(np.float32)
            m["b2b"] = ln2_b.reshape(1, D).astype(np.float32)
        in_maps.append(m)
    return in_maps, ln1_identity, ln2_identity


def kernel(target, source, W1, b1, W2, b2, ln1_g, ln1_b, ln2_g, ln2_b,
           _trace=False, _sim=False):
    args = [np.asarray(x, np.float32) for x in
            (target, source, W1, b1, W2, b2, ln1_g, ln1_b, ln2_g, ln2_b)]
    in_maps, ln1_id, ln2_id = _shard_inputs(*args)
    nc = _get_built(ln1_id, ln2_id)
    if _sim:
        from concourse.bass_interp import MultiCoreSim
        sim = MultiCoreSim(nc, num_cores=NC, require_finite=False,
                           require_nnan=False)
        for cid, core in sim.cores.items():
            for kname, v in in_maps[cid].items():
                core.tensor(kname)[:] = v
            if nc.partition_id_tensor is not None:
                core.tensor(nc.partition_id_tensor.name)[:] = np.array(
                    [[cid]], np.uint32)
        sim.simulate(check_with_hw=False)
        results = [{"out": np.asarray(sim.cores[c].tensor("out"))}
                   for c in range(NC)]
        exec_ns = None
    else:
        res = run_bass_kernel_spmd(nc, in_maps, core_ids=list(range(NC)),
                                   trace=_trace)
        results = res.results
        exec_ns = res.exec_time_ns
    out = np.empty((L, B, D), np.float32)
    for c in range(NC):
        b = c // 2
        lh = c % 2
        o = results[c]["out"]          # [128, TC, D]
        o = o.transpose(1, 0, 2).reshape(512, D)
        out[lh * 512:(lh + 1) * 512, b, :] = o
    kernel.last_exec_time_ns = exec_ns
    return out


# revision 19
# speedup vs baseline: 1.0621x; 1.0621x over previous
"""Trainium2 Bass kernel for nn_ArgExtractorLayer_35527969472569.

ProbSparse (Informer) cross-attention + FFN layer, distributed over 8
NeuronCores: attention is sharded by (batch, head-quad); a pairwise
AllToAll swaps token-halves so the FFN runs token-parallel.

Host-side work is limited to layout (slicing / transposing / dtype
casts) and input-independent constants (the fixed ProbSparse sampling
pattern of jax.random.key(42)); all input-dependent math runs on
device.
"""

import math
import numpy as np
import ml_dtypes

import concourse.bass as bass
import concourse.bacc as bacc
import concourse.tile as tile
from concourse import mybir
from concourse.bass_utils import run_bass_kernel_spmd

L, B, D = 1024, 4, 768
H, DH, DFF = 8, 96, 2048
U = min(int(5 * math.ceil(math.log(L))), L)  # 35
NC = 8
LN_EPS = 1e-5
SCALE = 1.0 / math.sqrt(DH)
QC = L // 128            # 8 query chunks per (b,h)
KC = L // 128            # 8 key chunks
DC = D // 128            # 6 d_model chunks
FC = DFF // 128          # 16 d_ff chunks
TC = 4                   # 4 token chunks of 128 per core (512 tokens)

f32 = mybir.dt.float32
f32r = mybir.dt.float32r
bf16 = mybir.dt.bfloat16
A = mybir.AluOpType
AF = mybir.ActivationFunctionType
X = mybir.AxisListType.X

_bf = ml_dtypes.bfloat16


def _sample_idx():
    """The constant ProbSparse sampling pattern (input-independent)."""
    import jax
    with jax.default_device(jax.devices("cpu")[0]):
        return np.asarray(jax.random.randint(jax.random.key(42), (L, U), 0, L))


def _constants():
    idx = _sample_idx()
    cnt = np.zeros((L, L), np.float32)
    for l in range(L):
        np.add.at(cnt[l], idx[l], 1.0)
    mask = np.where(cnt > 0, 0.0, -1e30).astype(np.float32)
    # mask_h[p, qc, j] = mask[qc*128+p, j]
    mask_h = np.ascontiguousarray(
        mask.reshape(QC, 128, L).transpose(1, 0, 2)).astype(_bf)
    # cntt_h[kc, lc, p, m] = cnt[lc*128+m, kc*128+p]  (CNT^T chunks)
    # cntt_h[p, kc, lc, m] = cnt[lc*128+m, kc*128+p]
    cntt_h = np.ascontiguousarray(
        cnt.T.reshape(KC, 128, QC, 128).transpose(1, 0, 2, 3)).astype(_bf)
    return mask_h, cntt_h


def build(ln1_identity: bool, ln2_identity: bool):
    nc = bacc.Bacc(None, target_bir_lowering=False, num_devices=NC)

    def inp(name, shape, dt):
        return nc.dram_tensor(name, shape, dt, kind="ExternalInput")

    qt_d = inp("qt", [4, DH, L], bf16)
    kt_d = inp("kt", [4, DH, L], bf16)
    qrow_d = inp("qrow", [128, 4, KC, DH], bf16)
    krr_d = inp("krr", [128, 4, KC, DH], bf16)
    krb_d = inp("krb", [128, 4, KC, DH], bf16)
    mask_d = inp("mask", [128, QC, L], bf16)
    cntt_d = inp("cntt", [128, KC, QC, 128], bf16)
    tgt_d = inp("tgt", [128, TC, D], f32)
    w1t_d = inp("w1t", [128, DC, DFF], bf16)
    b1c_d = inp("b1c", [128, FC], f32)
    w2t_d = inp("w2t", [128, FC, D], bf16)
    b2r_d = inp("b2r", [1, D], bf16)
    identb_d = inp("identb", [128, 128], bf16)
    onesr_d = inp("onesr", [1, 128], f32)       # row of ones (f32, exact bcast)
    onescol_d = inp("onescol", [128, 1], bf16)  # column of ones (colsum lhsT)
    onesbf_d = inp("onesbf", [1, 128], bf16)
    identf_d = inp("identf", [128, 128], f32)
    zlo_d = inp("zlo", [128, 1], f32)
    zhi_d = inp("zhi", [128, 1], f32)
    if not ln1_identity:
        g1b_d = inp("g1b", [1, D], f32)
        b1b_d = inp("b1b", [1, D], f32)
    if not ln2_identity:
        g2b_d = inp("g2b", [1, D], f32)
        b2b_d = inp("b2b", [1, D], f32)
    out_d = nc.dram_tensor("out", [128, TC, D], f32, kind="ExternalOutput")

    import contextlib
    with tile.TileContext(nc) as tc, contextlib.ExitStack() as ctx:
        singles = ctx.enter_context(tc.tile_pool(name="singles", bufs=1))
        dram = ctx.enter_context(tc.tile_pool(name="dram", bufs=1, space="DRAM"))

        identb = singles.tile([128, 128], bf16)
        nc.sync.dma_start(out=identb, in_=identb_d[:, :])
        identf = singles.tile([128, 128], f32)
        nc.sync.dma_start(out=identf, in_=identf_d[:, :])
        onesr = singles.tile([1, 128], f32)
        nc.sync.dma_start(out=onesr, in_=onesr_d[:, :])
        onescol = singles.tile([128, 1], bf16)
        nc.sync.dma_start(out=onescol, in_=onescol_d[:, :])
        onesbf = singles.tile([1, 128], bf16)
        nc.sync.dma_start(out=onesbf, in_=onesbf_d[:, :])
        b2r = singles.tile([1, D], bf16)
        nc.sync.dma_start(out=b2r, in_=b2r_d[:, :])
        b1c = singles.tile([128, FC], f32)
        nc.sync.dma_start(out=b1c, in_=b1c_d[:, :])
        w1t = singles.tile([128, DC, DFF], bf16)
        nc.scalar.dma_start(out=w1t, in_=w1t_d[:, :, :])
        w2t = singles.tile([128, FC, D], bf16)
        nc.scalar.dma_start(out=w2t, in_=w2t_d[:, :, :])
        krb = singles.tile([128, 4, KC, DH], bf16)
        nc.gpsimd.dma_start(out=krb, in_=krb_d[:, :, :, :])
        mask_t = singles.tile([128, QC, L], bf16)
        nc.gpsimd.dma_start(out=mask_t, in_=mask_d[:, :, :])
        cntt_sb = singles.tile([128, KC, QC, 128], bf16)
        nc.gpsimd.dma_start(out=cntt_sb, in_=cntt_d[:, :, :, :])
        zlo = singles.tile([128, 1], f32)
        nc.sync.dma_start(out=zlo, in_=zlo_d[:, :])
        zhi = singles.tile([128, 1], f32)
        nc.sync.dma_start(out=zhi, in_=zhi_d[:, :])
        epsb = singles.tile([128, 1], f32)
        nc.vector.memset(epsb, LN_EPS)

        ksum = singles.tile([128, QC, 4, DH], f32)   # Ksum = CNT @ K, all pairs
        meanvb = singles.tile([128, 4, DH], f32)     # mean of V rows, bcast

        # RS input shards: [token_half, head_slot 8, l_local, dh]
        ctxbuf = dram.tile([2, 8, 512, DH], bf16)
        rsout = dram.tile([8, 512, DH], bf16)
        m_scr = dram.tile([4, L], f32)

        # ---- Stage 0: Ksum (CNT @ K) and meanV, batched over the 4 pairs
        with tc.tile_pool(name="st0ps", bufs=2, space="PSUM") as st0ps, \
             tc.tile_pool(name="st0ps1", bufs=1, space="PSUM") as st0ps1, \
             tc.tile_pool(name="krrp", bufs=1) as krrp:
            krr = krrp.tile([128, 4, KC, DH], bf16)
            nc.gpsimd.dma_start(out=krr, in_=krr_d[:, :, :, :])

            mv_ps = st0ps1.tile([1, 4, DH], f32)
            for kc in range(KC):
                nc.tensor.matmul(mv_ps, onescol, krr[:, :, kc, :],
                                 start=(kc == 0), stop=(kc == KC - 1))
            mv_sb = singles.tile([1, 4, DH], f32)
            nc.scalar.activation(out=mv_sb, in_=mv_ps, func=AF.Copy,
                                 bias=0.0, scale=1.0 / L)
            mvb_ps = st0ps1.tile([128, 4, DH], f32)
            nc.tensor.matmul(mvb_ps, onesr, mv_sb, start=True, stop=True)
            nc.scalar.copy(meanvb, mvb_ps)

            for lc in range(QC):
                ks_ps = st0ps.tile([128, 4, DH], f32)
                for kc in range(KC):
                    nc.tensor.matmul(ks_ps, cntt_sb[:, kc, lc, :],
                                     krr[:, :, kc, :],
                                     start=(kc == 0), stop=(kc == KC - 1))
                nc.scalar.copy(ksum[:, lc, :, :], ks_ps)

        # ---- Stage 1: per-(b,h) attention
        with tc.tile_pool(name="pqk", bufs=2) as pqk, \
             tc.tile_pool(name="petr", bufs=2) as petr, \
             tc.tile_pool(name="pscr", bufs=2) as pscr, \
             tc.tile_pool(name="psml", bufs=2) as psml, \
             tc.tile_pool(name="pctx", bufs=2) as pctx, \
             tc.tile_pool(name="psA", bufs=2, space="PSUM") as psA, \
             tc.tile_pool(name="psU", bufs=2, space="PSUM") as psU:
            for p in range(4):
                qt_t = pqk.tile([DH, L], bf16, tag="qt")
                nc.sync.dma_start(out=qt_t, in_=qt_d[p, :, :])
                kt_t = pqk.tile([DH, L], bf16, tag="kt")
                nc.sync.dma_start(out=kt_t, in_=kt_d[p, :, :])
                qrow_t = pqk.tile([128, KC, DH], bf16, tag="qrow")
                nc.sync.dma_start(out=qrow_t, in_=qrow_d[:, p, :, :])

                etr = petr.tile([128, KC, L], bf16, tag="etr")   # E^T chunks
                mmax_t = psml.tile([128, QC], f32, tag="mmax")
                msum_t = psml.tile([128, QC], f32, tag="msum")
                esum_t = psml.tile([128, QC], f32, tag="esum")

                # S chunks: M-measure stats + softmax row sums
                for qc in range(QC):
                    s_ps = psA.tile([128, L], f32, tag="S")
                    for h in range(2):
                        nc.tensor.matmul(
                            s_ps[:, h * 512:(h + 1) * 512],
                            qt_t[:, qc * 128:(qc + 1) * 128],
                            kt_t[:, h * 512:(h + 1) * 512],
                            start=True, stop=True)
                    scr = pscr.tile([128, L], bf16, tag="scr")
                    nc.vector.tensor_tensor(scr, s_ps, mask_t[:, qc, :], A.add)
                    nc.vector.reduce_max(mmax_t[:, qc:qc + 1], scr, axis=X)
                    escr = pscr.tile([128, L], bf16, tag="escr")
                    nc.scalar.activation(out=escr, in_=s_ps,
                                         func=AF.Exp, bias=0.0, scale=SCALE,
                                         accum_out=esum_t[:, qc:qc + 1])
                    scr96 = pscr.tile([128, DH], f32, tag="scr96")
                    nc.vector.scalar_tensor_tensor(
                        out=scr96, in0=qrow_t[:, qc, :], scalar=1.0,
                        in1=ksum[:, qc, p, :], op0=A.bypass, op1=A.mult,
                        accum_out=msum_t[:, qc:qc + 1])

                # S^T chunks -> E^T directly (no PE transposes of E)
                for kc in range(KC):
                    st_ps = psA.tile([128, L], f32, tag="S")
                    for h in range(2):
                        nc.tensor.matmul(
                            st_ps[:, h * 512:(h + 1) * 512],
                            kt_t[:, kc * 128:(kc + 1) * 128],
                            qt_t[:, h * 512:(h + 1) * 512],
                            start=True, stop=True)
                    nc.scalar.activation(out=etr[:, kc, :], in_=st_ps,
                                         func=AF.Exp, bias=0.0, scale=SCALE)

                # M = mmax - msum/U ; exact f32 broadcast ; rank count
                m_t = psml.tile([128, QC], f32, tag="m")
                nc.vector.scalar_tensor_tensor(
                    out=m_t, in0=msum_t, scalar=-1.0 / U, in1=mmax_t,
                    op0=A.mult, op1=A.add)
                mdst = bass.AP(tensor=m_scr.tensor,
                               offset=m_scr.offset + p * L,
                               ap=[[1, 128], [128, QC]])
                nc.sync.dma_start(out=mdst, in_=m_t)
                mrow = psml.tile([1, L], f32, tag="mrow")
                nc.sync.dma_start(out=mrow, in_=m_scr[p, None, :])
                mb_ps = psA.tile([128, L], f32, tag="S")
                for h in range(2):
                    nc.tensor.matmul(mb_ps[:, h * 512:(h + 1) * 512], onesr,
                                     mrow[:, h * 512:(h + 1) * 512],
                                     start=True, stop=True)
                mb_sb = pscr.tile([128, L], f32, tag="mb")
                nc.scalar.copy(mb_sb, mb_ps)
                cnt_t = psml.tile([128, QC], f32, tag="cnt")
                for qc in range(QC):
                    scr2 = pscr.tile([128, L], bf16, tag="rcscr")
                    nc.vector.tensor_scalar(
                        out=scr2, in0=mb_sb, scalar1=m_t[:, qc:qc + 1],
                        scalar2=None, op0=A.is_gt, op1=A.add,
                        accum_out=cnt_t[:, qc:qc + 1])
                sel_t = psml.tile([128, QC], f32, tag="sel")
                nc.vector.tensor_scalar(out=sel_t, in0=cnt_t, scalar1=float(U),
                                        scalar2=None, op0=A.is_lt)
                nsel_t = psml.tile([128, QC], f32, tag="nsel")
                nc.vector.tensor_scalar(out=nsel_t, in0=sel_t, scalar1=-1.0,
                                        scalar2=-1.0, op0=A.mult, op1=A.subtract)
                recip_t = psml.tile([128, QC], f32, tag="recip")
                nc.vector.reciprocal(recip_t, esum_t)
                a_t = psml.tile([128, QC], f32, tag="a")
                nc.vector.tensor_tensor(a_t, sel_t, recip_t, A.mult)

                # PV in transposed orientation: updT[dh, l] accumulated over kc
                updt_full = psA.tile([128, L], f32, tag="S")
                updt_ps = updt_full[0:DH, :]
                for kc in range(KC):
                    for h in range(2):
                        nc.tensor.matmul(
                            updt_ps[:, h * 512:(h + 1) * 512],
                            krb[:, p, kc, :],
                            etr[:, kc, h * 512:(h + 1) * 512],
                            start=(kc == 0), stop=(kc == KC - 1))
                updt_sb = pscr.tile([DH, L], bf16, tag="updt")
                nc.scalar.copy(updt_sb, updt_ps)

                ctx_sb = pctx.tile([128, QC, DH], bf16, tag="ctx")
                for qc in range(QC):
                    u_ps = psU.tile([128, DH], bf16, tag="u")
                    nc.tensor.transpose(u_ps,
                                        updt_sb[:, qc * 128:(qc + 1) * 128],
                                        identb[0:DH, 0:DH])
                    u1 = pscr.tile([128, DH], f32, tag="u1")
                    nc.scalar.activation(out=u1, in_=u_ps, func=AF.Identity,
                                         bias=0.0, scale=a_t[:, qc:qc + 1])
                    nc.vector.scalar_tensor_tensor(
                        out=ctx_sb[:, qc, :], in0=meanvb[:, p, :],
                        scalar=nsel_t[:, qc:qc + 1], in1=u1,
                        op0=A.mult, op1=A.add)

                # masked copies: head slot p (low quad) and 4+p (high quad)
                ctx_lo = pctx.tile([128, QC, DH], bf16, tag="ctxlo")
                nc.vector.tensor_scalar(out=ctx_lo, in0=ctx_sb,
                                        scalar1=zlo[:, 0:1], scalar2=None,
                                        op0=A.mult)
                ctx_hi = pctx.tile([128, QC, DH], bf16, tag="ctxhi")
                nc.vector.tensor_scalar(out=ctx_hi, in0=ctx_sb,
                                        scalar1=zhi[:, 0:1], scalar2=None,
                                        op0=A.mult)
                for tl, slot in ((ctx_lo, p), (ctx_hi, 4 + p)):
                    for m in range(2):
                        cdst = bass.AP(
                            tensor=ctxbuf.tensor,
                            offset=(ctxbuf.offset + m * 8 * 512 * DH
                                    + slot * 512 * DH),
                            ap=[[DH, 128], [128 * DH, 4], [1, DH]])
                        nc.sync.dma_start(out=cdst,
                                          in_=tl[:, m * 4:(m + 1) * 4, :])

        # ---- Stage 2: pairwise ReduceScatter (heads are feature-disjoint,
        # so the add concatenates this core's quad with its partner's).
        nc.gpsimd.collective_compute(
            "ReduceScatter", A.add,
            replica_groups=[[0, 1], [2, 3], [4, 5], [6, 7]],
            ins=[ctxbuf.opt()], outs=[rsout.opt()])
        # rsout[h, l_local, dh]: all 8 heads for MY 512 tokens.

        # ---- Stage 3: token-parallel LN1 -> FFN -> LN2
        with tc.tile_pool(name="ffn", bufs=2) as ffn, \
             tc.tile_pool(name="ffn1", bufs=1) as ffn1, \
             tc.tile_pool(name="ptiny", bufs=4) as ptiny, \
             tc.tile_pool(name="psN", bufs=1, space="PSUM") as psN, \
             tc.tile_pool(name="psH", bufs=2, space="PSUM") as psH, \
             tc.tile_pool(name="psP", bufs=2, space="PSUM") as psP:
            n1_all = ffn1.tile([128, TC, D], f32)
            n1t_sb = ffn1.tile([128, DC, TC, 128], bf16)
            h1t_sb = ffn1.tile([128, FC, 512], bf16)
            tgt_t = ffn1.tile([128, TC, D], f32)
            nc.gpsimd.dma_start(out=tgt_t, in_=tgt_d[:, :, :])
            gb1 = bb1 = gb2 = bb2 = None
            if not ln1_identity:
                gb1 = ffn1.tile([128, D], f32)
                nc.sync.dma_start(out=gb1, in_=g1b_d[:, :].broadcast_to([128, D]))
                bb1 = ffn1.tile([128, D], f32)
                nc.sync.dma_start(out=bb1, in_=b1b_d[:, :].broadcast_to([128, D]))
            if not ln2_identity:
                gb2 = ffn1.tile([128, D], f32)
                nc.sync.dma_start(out=gb2, in_=g2b_d[:, :].broadcast_to([128, D]))
                bb2 = ffn1.tile([128, D], f32)
                nc.sync.dma_start(out=bb2, in_=b2b_d[:, :].broadcast_to([128, D]))

            def layer_norm(x_t, out_ap, gb, bb):
                """LN over the free dim (D) with per-partition stats."""
                sum_t = ptiny.tile([128, 1], f32, tag="t1")
                nc.vector.reduce_sum(sum_t, x_t, axis=X)
                sq_scr = ffn.tile([128, D], f32, tag="sqscr")
                ssq_t = ptiny.tile([128, 1], f32, tag="t2")
                nc.scalar.activation(out=sq_scr, in_=x_t, func=AF.Square,
                                     bias=0.0, scale=1.0, accum_out=ssq_t)
                mu_t = ptiny.tile([128, 1], f32, tag="t3")
                nc.vector.tensor_scalar(out=mu_t, in0=sum_t, scalar1=1.0 / D,
                                        scalar2=None, op0=A.mult)
                musq_t = ptiny.tile([128, 1], f32, tag="t4")
                nc.vector.tensor_tensor(musq_t, mu_t, mu_t, A.mult)
                var_t = ptiny.tile([128, 1], f32, tag="t5")
                nc.vector.scalar_tensor_tensor(
                    out=var_t, in0=ssq_t, scalar=1.0 / D, in1=musq_t,
                    op0=A.mult, op1=A.subtract)
                sd_t = ptiny.tile([128, 1], f32, tag="t6")
                nc.scalar.activation(out=sd_t, in_=var_t, func=AF.Sqrt,
                                     bias=epsb[:, 0:1], scale=1.0)
                rstd_t = ptiny.tile([128, 1], f32, tag="t7")
                nc.vector.reciprocal(rstd_t, sd_t)
                nb_t = ptiny.tile([128, 1], f32, tag="t8")
                nc.vector.scalar_tensor_tensor(
                    out=nb_t, in0=mu_t, scalar=-1.0, in1=rstd_t,
                    op0=A.mult, op1=A.mult)
                if gb is None:
                    nc.scalar.activation(out=out_ap, in_=x_t, func=AF.Identity,
                                         bias=nb_t, scale=rstd_t)
                else:
                    xh = ffn.tile([128, D], f32, tag="xhat")
                    nc.scalar.activation(out=xh, in_=x_t, func=AF.Identity,
                                         bias=nb_t, scale=rstd_t)
                    xg = ffn.tile([128, D], f32, tag="xg")
                    nc.vector.tensor_tensor(xg, xh, gb, A.mult)
                    nc.vector.tensor_tensor(out_ap, xg, bb, A.add)

            for tc4 in range(TC):
                att_t = ffn.tile([128, D], bf16, tag="att")
                asrc = bass.AP(tensor=rsout.tensor,
                               offset=rsout.offset + tc4 * 128 * DH,
                               ap=[[DH, 128], [512 * DH, 8], [1, DH]])
                nc.sync.dma_start(out=att_t, in_=asrc)
                x_t = ffn.tile([128, D], f32, tag="x")
                nc.vector.tensor_tensor(x_t, tgt_t[:, tc4, :], att_t, A.add)
                layer_norm(x_t, n1_all[:, tc4, :], gb1, bb1)
                n1t_ps = psN.tile([128, DC, 128], f32, tag="n1t")
                for dc in range(DC):
                    nc.tensor.transpose(n1t_ps[:, dc, :],
                                        n1_all[:, tc4, dc * 128:(dc + 1) * 128],
                                        identf)
                nc.scalar.copy(n1t_sb[:, :, tc4, :], n1t_ps)

            for fc in range(FC):
                h_ps = psH.tile([128, 512], f32, tag="h")
                for dc in range(DC):
                    nc.tensor.matmul(h_ps, w1t[:, dc, fc * 128:(fc + 1) * 128],
                                     n1t_sb[:, dc, :, :],
                                     start=(dc == 0), stop=(dc == DC - 1))
                nc.scalar.activation(out=h1t_sb[:, fc, :], in_=h_ps, func=AF.Relu,
                                     bias=b1c[:, fc:fc + 1], scale=1.0)

            for tc4 in range(TC):
                pr_ps = psP.tile([128, D], f32, tag="pr")
                for lo, hi in ((0, 512), (512, D)):
                    for fc in range(FC):
                        nc.tensor.matmul(pr_ps[:, lo:hi],
                                         h1t_sb[:, fc, tc4 * 128:(tc4 + 1) * 128],
                                         w2t[:, fc, lo:hi],
                                         start=(fc == 0), stop=False)
                    nc.tensor.matmul(pr_ps[:, lo:hi], onesbf, b2r[:, lo:hi],
                                     start=False, stop=True)
                x2_t = ffn.tile([128, D], f32, tag="x2")
                nc.vector.tensor_tensor(x2_t, n1_all[:, tc4, :], pr_ps, A.add)
                out_sb = ffn.tile([128, D], f32, tag="osb")
                layer_norm(x2_t, out_sb, gb2, bb2)
                nc.sync.dma_start(out=out_d[:, tc4, :], in_=out_sb)

    nc.finalize()
    return nc


# ---------------------------------------------------------------------------
# Host-side sharding + execution

_cache = {}


def _get_built(ln1_identity, ln2_identity):
    key = (ln1_identity, ln2_identity)
    if key not in _cache:
        _cache[key] = build(ln1_identity, ln2_identity)
    return _cache[key]


def _shard_inputs(target, source, W1, b1, W2, b2, ln1_g, ln1_b, ln2_g, ln2_b):
    mask_h, cntt_h = _constants()
    identb = np.eye(128, dtype=np.float32).astype(_bf)
    identf = np.eye(128, dtype=np.float32)
    onesr = np.ones((1, 128), np.float32)
    onescol = np.ones((128, 1), np.float32).astype(_bf)
    onesbf = np.ones((1, 128), np.float32).astype(_bf)
    w1t = np.ascontiguousarray(W1.T.reshape(DC, 128, DFF).transpose(1, 0, 2)).astype(_bf)
    b1c = np.ascontiguousarray(b1.reshape(FC, 128).T).astype(np.float32)
    w2t = np.ascontiguousarray(W2.T.reshape(FC, 128, D).transpose(1, 0, 2)).astype(_bf)
    b2r = b2.reshape(1, D).astype(_bf)

    # target/source as [B, H, L, Dh] views
    tr = target.reshape(L, B, H, DH)
    sr = source.reshape(L, B, H, DH)

    ln1_identity = bool(np.all(ln1_g == 1.0) and np.all(ln1_b == 0.0))
    ln2_identity = bool(np.all(ln2_g == 1.0) and np.all(ln2_b == 0.0))

    in_maps = []
    for c in range(NC):
        b = c // 2
        hq = c % 2
        lh = c % 2
        hs = slice(4 * hq, 4 * hq + 4)
        q = np.ascontiguousarray(tr[:, b, hs, :])      # [L, 4, DH]
        k = np.ascontiguousarray(sr[:, b, hs, :])      # [L, 4, DH]
        qt = np.ascontiguousarray(q.transpose(1, 2, 0)).astype(_bf)   # [4, DH, L]
        kt = np.ascontiguousarray(k.transpose(1, 2, 0)).astype(_bf)   # [4, DH, L]
        # [128, 4, KC, DH]: [pp, pair, kc, dh] with l = kc*128+pp
        qrow = np.ascontiguousarray(
            q.reshape(KC, 128, 4, DH).transpose(1, 2, 0, 3))
        krow = np.ascontiguousarray(
            k.reshape(KC, 128, 4, DH).transpose(1, 2, 0, 3))
        tgt = np.ascontiguousarray(
            target[lh * 512:(lh + 1) * 512, b, :]
            .reshape(TC, 128, D).transpose(1, 0, 2))
        zlo = np.full((128, 1), 1.0 if hq == 0 else 0.0, np.float32)
        zhi = np.full((128, 1), 0.0 if hq == 0 else 1.0, np.float32)
        m = dict(qt=qt, kt=kt, qrow=qrow.astype(_bf), krr=krow.astype(_bf),
                 krb=krow.astype(_bf),
                 mask=mask_h, cntt=cntt_h, tgt=tgt, w1t=w1t, b1c=b1c,
                 w2t=w2t, b2r=b2r, identb=identb, identf=identf,
                 onesr=onesr, onescol=onescol, onesbf=onesbf,
                 zlo=zlo, zhi=zhi)
        if not ln1_identity:
            m["g1b"] = ln1_g.reshape(1, D).astype(np.float32)
            m["b1b"] = ln1_b.reshape(1, D).astype(np.float32)
        if not ln2_identity:
            m["g2b"] = ln2_g.reshape(1, D).astype(np.float32)
            m["b2b"] = ln2_b.reshape(1, D).astype(np.float32)
        in_maps.append(m)
    return in_maps, ln1_identity, ln2_identity


def kernel(target, source, W1, b1, W2, b2, ln1_g, ln1_b, ln2_g, ln2_b,
           _trace=False, _sim=False):
    args = [np.asarray(x, np.float32) for x in
            (target, source, W1, b1, W2, b2, ln1_g, ln1_b, ln2_g, ln2_b)]
    in_maps, ln1_id, ln2_id = _shard_inputs(*args)
    nc = _get_built(ln1_id, ln2_id)
    if _sim:
        from concourse.bass_interp import MultiCoreSim
        sim = MultiCoreSim(nc, num_cores=NC, require_finite=False,
                           require_nnan=False)
        for cid, core in sim.cores.items():
            for kname, v in in_maps[cid].items():
                core.tensor(kname)[:] = v
            if nc.partition_id_tensor is not None:
                core.tensor(nc.partition_id_tensor.name)[:] = np.array(
                    [[cid]], np.uint32)
        sim.simulate(check_with_hw=False)
        results = [{"out": np.asarray(sim.cores[c].tensor("out"))}
                   for c in range(NC)]
        exec_ns = None
    else:
        res = run_bass_kernel_spmd(nc, in_maps, core_ids=list(range(NC)),
                                   trace=_trace)
        results = res.results
        exec_ns = res.exec_time_ns
    out = np.empty((L, B, D), np.float32)
    for c in range(NC):
        b = c // 2
        lh = c % 2
        o = results[c]["out"]          # [128, TC, D]
        o = o.transpose(1, 0, 2).reshape(512, D)
        out[lh * 512:(lh + 1) * 512, b, :] = o
    kernel.last_exec_time_ns = exec_ns
    return out
